# revision 6
# baseline (speedup 1.0000x reference)
"""flash_wave CA kernel for Trainium2 (Bass/Tile).

Device computes T_CHUNK un-frozen CA steps (fp16) and streams every state to
DRAM. The early-exit freeze of the reference is applied host-side: the first
step t* where the wave reaches (ex,ey,ez) is detected from the returned
frames; frames after t*+1 are provably identical to frame t*+1 (the reference
freezes phi), so the tail is replicated on host and any device frames beyond
t*+1 (computed without freezing) are discarded.

Device layout (single core): partition p = xq*32 + y (xq = x>>3),
free dim = [ch 6][xl 8][z 32] (fp16). Shifts:
  x: free-dim xl windows + quadrant-aligned partition-crossing copies (DVE)
  y: PE shift-matrix matmul (boundary mask baked into the matrix) -> PSUM -> ACT
  z: free-dim windows (DVE)
Clip: pn >= 0 always, so clip(x) = min(x, 1.0), applied once to phi_out
before the shifts (clip commutes with shifts).
"""
import numpy as np

GRID = 32
CH = 6
RING = 16
T_CHUNK = 100

_build_cache = {}


def _build(T):
    if T in _build_cache:
        return _build_cache[T]
    import concourse.bacc as bacc
    import concourse.mybir as mybir
    from concourse.tile import TileContext

    F16 = mybir.dt.float16
    F32 = mybir.dt.float32
    OP = mybir.AluOpType

    nc = bacc.Bacc("TRN2", target_bir_lowering=False, debug=False)
    d_in = nc.dram_tensor("d_in", [128, CH * CH * 256], F16, kind="ExternalInput")
    phi0 = nc.dram_tensor("phi0", [128, CH * 256], F16, kind="ExternalInput")
    smat = nc.dram_tensor("smat", [128, 640], F16, kind="ExternalInput")
    frames = nc.dram_tensor("frames", [T, 128, CH * 256], F16, kind="ExternalOutput")

    D = nc.alloc_sbuf_tensor("D", [128, CH * CH * 256], F16)
    S = nc.alloc_sbuf_tensor("S", [128, 640], F16)
    ring = [nc.alloc_sbuf_tensor(f"ring{i}", [128, CH * 256], F16) for i in range(RING)]
    prod = nc.alloc_sbuf_tensor("prod", [128, CH * CH * 256], F16)
    t3 = nc.alloc_sbuf_tensor("t3", [128, CH * 3 * 256], F16)
    u = nc.alloc_sbuf_tensor("u", [128, CH * 256], F16)
    ta = nc.alloc_sbuf_tensor("ta", [128, CH * 256], F16)
    po = nc.alloc_sbuf_tensor("po", [128, CH * 256], F16)
    cl = nc.alloc_sbuf_tensor("cl", [128, CH * 256], F16)
    ps0 = nc.alloc_psum_tensor("ps0", [128, 256], F32)
    ps1 = nc.alloc_psum_tensor("ps1", [128, 256], F32)
    ps2 = nc.alloc_psum_tensor("ps2", [128, 256], F32)
    ps3 = nc.alloc_psum_tensor("ps3", [128, 256], F32)

    with TileContext(nc):
        nc.sync.dma_start(D[:, :], d_in[:, :])
        nc.sync.dma_start(ring[RING - 1][:, :], phi0[:, :])
        nc.sync.dma_start(S[:, :], smat[:, :])

        D4 = D[:, :].rearrange("p (o i c) -> p o i c", o=CH, i=CH, c=256)
        t34 = t3[:, :].rearrange("p (o j c) -> p o j c", o=CH, j=3, c=256)
        u3 = u[:, :].rearrange("p (o c) -> p o c", o=CH, c=256)
        ta3 = ta[:, :].rearrange("p (o c) -> p o c", o=CH, c=256)
        po3 = po[:, :].rearrange("p (o c) -> p o c", o=CH, c=256)
        prod4 = prod[:, :].rearrange("p (o i c) -> p o i c", o=CH, i=CH, c=256)
        cl4 = cl[:, :].rearrange("p (o xl z) -> p o xl z", o=CH, xl=8, z=GRID)
        po4 = po[:, :].rearrange("p (o xl z) -> p o xl z", o=CH, xl=8, z=GRID)
        cl3 = cl[:, :].rearrange("p (o c) -> p o c", o=CH, c=256)

        for t in range(T):
            prev = ring[(t + RING - 1) % RING]
            nxt = ring[t % RING]
            prev3 = prev[:, :].rearrange("p (i c) -> p i c", i=CH, c=256)
            phi_a = prev3[:, 4:6, :].unsqueeze(1).to_broadcast((128, CH, 2, 256))
            phi_c1 = prev3[:, 0:2, :].unsqueeze(1).to_broadcast((128, CH, 2, 256))
            phi_c2 = prev3[:, 2:4, :].unsqueeze(1).to_broadcast((128, CH, 2, 256))
            # mul over i in {4,5} first: its inputs come from DVE's own z-shift
            # writes, so it overlaps the ACT ch0..3 copies of the previous step
            nc.vector.tensor_tensor(prod4[:, :, 4:6, :], D4[:, :, 4:6, :], phi_a, op=OP.mult)
            nc.vector.tensor_tensor(ta3, prod4[:, :, 4, :], prod4[:, :, 5, :], op=OP.add)
            nc.vector.tensor_tensor(prod4[:, :, 0:2, :], D4[:, :, 0:2, :], phi_c1, op=OP.mult)
            nc.vector.tensor_tensor(
                t34[:, :, 0, :], prod4[:, :, 0, :], prod4[:, :, 1, :], op=OP.add
            )
            nc.vector.tensor_tensor(prod4[:, :, 2:4, :], D4[:, :, 2:4, :], phi_c2, op=OP.mult)
            nc.vector.tensor_tensor(
                t34[:, :, 1, :], prod4[:, :, 2, :], prod4[:, :, 3, :], op=OP.add
            )
            nc.vector.tensor_tensor(u3, t34[:, :, 0, :], t34[:, :, 1, :], op=OP.add)
            nc.vector.tensor_tensor(po3, u3, ta3, op=OP.add)
            nc.vector.tensor_scalar_min(cl[:, 0 : 4 * 256], po[:, 0 : 4 * 256], 1.0)

            nxt4 = nxt[:, :].rearrange("p (o xl z) -> p o xl z", o=CH, xl=8, z=GRID)
            nc.gpsimd.memset(nxt[:, :], 0.0)
            # ch0 (+x): free offset via identity matmul + quadrant-crossing matrix
            nc.tensor.matmul(ps0[:, 32:256], S[:, 512:640], cl3[:, 0, 0:224], start=True, stop=True)
            nc.tensor.matmul(ps0[:, 0:32], S[:, 256:384], cl3[:, 0, 224:256], start=True, stop=True)
            # ch1 (-x)
            nc.tensor.matmul(ps1[:, 0:224], S[:, 512:640], cl3[:, 1, 32:256], start=True, stop=True)
            nc.tensor.matmul(ps1[:, 224:256], S[:, 384:512], cl3[:, 1, 0:32], start=True, stop=True)
            # ch2 (+y), ch3 (-y)
            nc.tensor.matmul(ps2[:, :], S[:, 0:128], cl3[:, 2, :], start=True, stop=True)
            nc.tensor.matmul(ps3[:, :], S[:, 128:256], cl3[:, 3, :], start=True, stop=True)
            nc.scalar.copy(nxt4[:, 0, 1:8, :].rearrange("p a b -> p (a b)"), ps0[:, 32:256])
            nc.scalar.copy(nxt4[:, 0, 0, :], ps0[:, 0:32])
            nc.scalar.copy(nxt4[:, 1, 0:7, :].rearrange("p a b -> p (a b)"), ps1[:, 0:224])
            nc.scalar.copy(nxt4[:, 1, 7, :], ps1[:, 224:256])
            nc.scalar.copy(nxt4[:, 2, :, :].rearrange("p a b -> p (a b)"), ps2[:, :])
            nc.scalar.copy(nxt4[:, 3, :, :].rearrange("p a b -> p (a b)"), ps3[:, :])
            nc.vector.tensor_scalar_min(nxt4[:, 4, :, 1:GRID], po4[:, 4, :, 0 : GRID - 1], 1.0)
            nc.vector.tensor_scalar_min(nxt4[:, 5, :, 0 : GRID - 1], po4[:, 5, :, 1:GRID], 1.0)

            nc.sync.dma_start(frames[t], nxt[:, :])
    nc.compile()
    _build_cache[T] = nc
    return nc


def _arrange_D(Dact):
    a = Dact.reshape(CH, CH, 4, 8, GRID, GRID)  # o i xq xl y z
    a = a.transpose(2, 4, 0, 1, 3, 5).reshape(128, CH * CH * 256)
    return np.ascontiguousarray(a).astype(np.float16)


def _arrange_state(phi):
    a = phi.reshape(CH, 4, 8, GRID, GRID).transpose(1, 3, 0, 2, 4).reshape(128, CH * 256)
    return np.ascontiguousarray(a).astype(np.float16)


def _unarrange_frames(fr):  # [T,128,1536] -> [T,6,32,32,32] float32
    T = fr.shape[0]
    return (
        fr.reshape(T, 4, GRID, CH, 8, GRID)
        .transpose(0, 3, 1, 4, 2, 5)
        .reshape(T, CH, GRID, GRID, GRID)
        .astype(np.float32)
    )


def _make_smat():
    m = np.arange(128)
    S_up = ((m[None, :] - 1 == m[:, None]) & (m[None, :] % 32 != 0)).astype(np.float16)
    S_dn = ((m[None, :] + 1 == m[:, None]) & (m[None, :] % 32 != 31)).astype(np.float16)
    Sx_up = (m[:, None] == m[None, :] - 32).astype(np.float16)
    Sx_dn = (m[:, None] == m[None, :] + 32).astype(np.float16)
    I = np.eye(128, dtype=np.float16)
    return np.concatenate([S_up, S_dn, Sx_up, Sx_dn, I], axis=1)


def _run_chunk(nc, ins, retries=3):
    from concourse.bass_utils import run_bass_kernel_spmd

    last = None
    for _ in range(retries):
        try:
            res = run_bass_kernel_spmd(nc, [ins], core_ids=[0])
            return res.results[0]["frames"]
        except Exception as e:  # transient NRT_EXEC_UNIT_UNRECOVERABLE seen on this setup
            last = e
    raise last


def kernel(D, sx, sy, sz, ex, ey, ez, max_iterations):
    D = np.asarray(D, dtype=np.float32)
    sx, sy, sz = int(sx), int(sy), int(sz)
    ex, ey, ez = int(ex), int(ey), int(ez)
    T_total = int(max_iterations)

    phi0 = np.zeros((CH, GRID, GRID, GRID), np.float32)
    phi0[:, sx, sy, sz] = 1.0

    d_arr = _arrange_D(D + np.float32(0.95))
    smat = _make_smat()

    out = np.empty((T_total, CH, GRID, GRID, GRID), np.float32)
    out[0] = phi0

    state = phi0
    base = 0  # number of steps already recorded beyond frame 0 -> out[0..base]
    # out[k] for k <= base is final. Loop until frozen or all frames produced.
    while base < T_total - 1:
        T = min(T_CHUNK, T_total - 1 - base)
        nc = _build(T)
        ins = {"d_in": d_arr, "phi0": _arrange_state(state), "smat": smat}
        fr = np.asarray(_run_chunk(nc, ins))
        frames = _unarrange_frames(fr)  # frames[j] = state after base+j+1 steps
        # detect freeze: reference freezes once sum(nxt[:, ex, ey, ez]) > 0.01.
        # nxt at global step t is out[t+1]; first such t = t*; frames beyond
        # t*+1 equal frame t*+1.
        sums = frames[:, :, ex, ey, ez].sum(axis=1)  # global steps base+1 .. base+T
        hit = np.nonzero(sums > 0.01)[0]
        if hit.size:
            tstar_plus1 = base + 1 + int(hit[0])  # global index of frozen frame
            n_keep = min(tstar_plus1 - base, T)
            out[base + 1 : base + 1 + n_keep] = frames[:n_keep]
            out[tstar_plus1 + 1 :] = out[tstar_plus1]
            return out
        out[base + 1 : base + 1 + T] = frames
        state = frames[T - 1]
        base += T
    return out


# revision 7
# speedup vs baseline: 1.0008x; 1.0008x over previous
"""flash_wave CA kernel for Trainium2 (Bass/Tile).

Device computes T_CHUNK un-frozen CA steps (fp16) and streams every state to
DRAM. The early-exit freeze of the reference is applied host-side: the first
step t* where the wave reaches (ex,ey,ez) is detected from the returned
frames; frames after t*+1 are provably identical to frame t*+1 (the reference
freezes phi), so the tail is replicated on host and any device frames beyond
t*+1 (computed without freezing) are discarded.

Device layout (single core): partition p = xq*32 + y (xq = x>>3),
free dim = [ch 6][xl 8][z 32] (fp16). Shifts:
  x: free-dim xl windows + quadrant-aligned partition-crossing copies (DVE)
  y: PE shift-matrix matmul (boundary mask baked into the matrix) -> PSUM -> ACT
  z: free-dim windows (DVE)
Clip: pn >= 0 always, so clip(x) = min(x, 1.0), applied once to phi_out
before the shifts (clip commutes with shifts).
"""
import numpy as np

GRID = 32
CH = 6
RING = 16
T_CHUNK = 100

_build_cache = {}


def _build(T):
    if T in _build_cache:
        return _build_cache[T]
    import concourse.bacc as bacc
    import concourse.mybir as mybir
    from concourse.tile import TileContext

    F16 = mybir.dt.float16
    F32 = mybir.dt.float32
    OP = mybir.AluOpType

    nc = bacc.Bacc("TRN2", target_bir_lowering=False, debug=False)
    d_in = nc.dram_tensor("d_in", [128, CH * CH * 256], F16, kind="ExternalInput")
    phi0 = nc.dram_tensor("phi0", [128, CH * 256], F16, kind="ExternalInput")
    smat = nc.dram_tensor("smat", [128, 640], F16, kind="ExternalInput")
    frames = nc.dram_tensor("frames", [T, 128, CH * 256], F16, kind="ExternalOutput")

    D = nc.alloc_sbuf_tensor("D", [128, CH * CH * 256], F16)
    S = nc.alloc_sbuf_tensor("S", [128, 640], F16)
    ring = [nc.alloc_sbuf_tensor(f"ring{i}", [128, CH * 256], F16) for i in range(RING)]
    prod = nc.alloc_sbuf_tensor("prod", [128, CH * CH * 256], F16)
    t3 = nc.alloc_sbuf_tensor("t3", [128, CH * 3 * 256], F16)
    u = nc.alloc_sbuf_tensor("u", [128, CH * 256], F16)
    ta = nc.alloc_sbuf_tensor("ta", [128, CH * 256], F16)
    po = nc.alloc_sbuf_tensor("po", [128, CH * 256], F16)
    cl = nc.alloc_sbuf_tensor("cl", [128, CH * 256], F16)
    ps0 = nc.alloc_psum_tensor("ps0", [128, 256], F32)
    ps1 = nc.alloc_psum_tensor("ps1", [128, 256], F32)
    ps2 = nc.alloc_psum_tensor("ps2", [128, 256], F32)
    ps3 = nc.alloc_psum_tensor("ps3", [128, 256], F32)

    with TileContext(nc):
        nc.sync.dma_start(D[:, :], d_in[:, :])
        nc.sync.dma_start(ring[RING - 1][:, :], phi0[:, :])
        nc.sync.dma_start(S[:, :], smat[:, :])

        D4 = D[:, :].rearrange("p (o i c) -> p o i c", o=CH, i=CH, c=256)
        t34 = t3[:, :].rearrange("p (o j c) -> p o j c", o=CH, j=3, c=256)
        u3 = u[:, :].rearrange("p (o c) -> p o c", o=CH, c=256)
        ta3 = ta[:, :].rearrange("p (o c) -> p o c", o=CH, c=256)
        po3 = po[:, :].rearrange("p (o c) -> p o c", o=CH, c=256)
        prod4 = prod[:, :].rearrange("p (o i c) -> p o i c", o=CH, i=CH, c=256)
        cl4 = cl[:, :].rearrange("p (o xl z) -> p o xl z", o=CH, xl=8, z=GRID)
        po4 = po[:, :].rearrange("p (o xl z) -> p o xl z", o=CH, xl=8, z=GRID)
        cl3 = cl[:, :].rearrange("p (o c) -> p o c", o=CH, c=256)

        for t in range(T):
            prev = ring[(t + RING - 1) % RING]
            nxt = ring[t % RING]
            prev3 = prev[:, :].rearrange("p (i c) -> p i c", i=CH, c=256)
            phi_a = prev3[:, 4:6, :].unsqueeze(1).to_broadcast((128, CH, 2, 256))
            phi_c = prev3[:, 0:4, :].unsqueeze(1).to_broadcast((128, CH, 4, 256))
            # mul over i in {4,5} first: its inputs come from DVE's own z-shift
            # writes, so it overlaps the ACT ch0..3 copies of the previous step
            nc.vector.tensor_tensor(prod4[:, :, 4:6, :], D4[:, :, 4:6, :], phi_a, op=OP.mult)
            nc.vector.tensor_tensor(ta3, prod4[:, :, 4, :], prod4[:, :, 5, :], op=OP.add)
            nc.vector.tensor_tensor(prod4[:, :, 0:4, :], D4[:, :, 0:4, :], phi_c, op=OP.mult)
            nc.vector.tensor_tensor(
                t34[:, :, 0:2, :], prod4[:, :, 0:4:2, :], prod4[:, :, 1:4:2, :], op=OP.add
            )
            nc.vector.tensor_tensor(u3, t34[:, :, 0, :], t34[:, :, 1, :], op=OP.add)
            nc.vector.tensor_tensor(po3, u3, ta3, op=OP.add)
            nc.vector.tensor_scalar_min(cl[:, 0 : 4 * 256], po[:, 0 : 4 * 256], 1.0)

            nxt4 = nxt[:, :].rearrange("p (o xl z) -> p o xl z", o=CH, xl=8, z=GRID)
            nc.gpsimd.memset(nxt[:, :], 0.0)
            # ch0 (+x): free offset via identity matmul + quadrant-crossing matrix
            nc.tensor.matmul(ps0[:, 32:256], S[:, 512:640], cl3[:, 0, 0:224], start=True, stop=True)
            nc.tensor.matmul(ps0[:, 0:32], S[:, 256:384], cl3[:, 0, 224:256], start=True, stop=True)
            # ch1 (-x)
            nc.tensor.matmul(ps1[:, 0:224], S[:, 512:640], cl3[:, 1, 32:256], start=True, stop=True)
            nc.tensor.matmul(ps1[:, 224:256], S[:, 384:512], cl3[:, 1, 0:32], start=True, stop=True)
            # ch2 (+y), ch3 (-y)
            nc.tensor.matmul(ps2[:, :], S[:, 0:128], cl3[:, 2, :], start=True, stop=True)
            nc.tensor.matmul(ps3[:, :], S[:, 128:256], cl3[:, 3, :], start=True, stop=True)
            nc.scalar.copy(nxt4[:, 0, 1:8, :].rearrange("p a b -> p (a b)"), ps0[:, 32:256])
            nc.scalar.copy(nxt4[:, 0, 0, :], ps0[:, 0:32])
            nc.scalar.copy(nxt4[:, 1, 0:7, :].rearrange("p a b -> p (a b)"), ps1[:, 0:224])
            nc.scalar.copy(nxt4[:, 1, 7, :], ps1[:, 224:256])
            nc.scalar.copy(nxt4[:, 2, :, :].rearrange("p a b -> p (a b)"), ps2[:, :])
            nc.scalar.copy(nxt4[:, 3, :, :].rearrange("p a b -> p (a b)"), ps3[:, :])
            nc.vector.tensor_scalar_min(nxt4[:, 4, :, 1:GRID], po4[:, 4, :, 0 : GRID - 1], 1.0)
            nc.vector.tensor_scalar_min(nxt4[:, 5, :, 0 : GRID - 1], po4[:, 5, :, 1:GRID], 1.0)

            nc.sync.dma_start(frames[t], nxt[:, :])
    nc.compile()
    _build_cache[T] = nc
    return nc


def _arrange_D(Dact):
    a = Dact.reshape(CH, CH, 4, 8, GRID, GRID)  # o i xq xl y z
    a = a.transpose(2, 4, 0, 1, 3, 5).reshape(128, CH * CH * 256)
    return np.ascontiguousarray(a).astype(np.float16)


def _arrange_state(phi):
    a = phi.reshape(CH, 4, 8, GRID, GRID).transpose(1, 3, 0, 2, 4).reshape(128, CH * 256)
    return np.ascontiguousarray(a).astype(np.float16)


def _unarrange_frames(fr):  # [T,128,1536] -> [T,6,32,32,32] float32
    T = fr.shape[0]
    return (
        fr.reshape(T, 4, GRID, CH, 8, GRID)
        .transpose(0, 3, 1, 4, 2, 5)
        .reshape(T, CH, GRID, GRID, GRID)
        .astype(np.float32)
    )


def _make_smat():
    m = np.arange(128)
    S_up = ((m[None, :] - 1 == m[:, None]) & (m[None, :] % 32 != 0)).astype(np.float16)
    S_dn = ((m[None, :] + 1 == m[:, None]) & (m[None, :] % 32 != 31)).astype(np.float16)
    Sx_up = (m[:, None] == m[None, :] - 32).astype(np.float16)
    Sx_dn = (m[:, None] == m[None, :] + 32).astype(np.float16)
    I = np.eye(128, dtype=np.float16)
    return np.concatenate([S_up, S_dn, Sx_up, Sx_dn, I], axis=1)


def _run_chunk(nc, ins, retries=3):
    from concourse.bass_utils import run_bass_kernel_spmd

    last = None
    for _ in range(retries):
        try:
            res = run_bass_kernel_spmd(nc, [ins], core_ids=[0])
            return res.results[0]["frames"]
        except Exception as e:  # transient NRT_EXEC_UNIT_UNRECOVERABLE seen on this setup
            last = e
    raise last


def kernel(D, sx, sy, sz, ex, ey, ez, max_iterations):
    D = np.asarray(D, dtype=np.float32)
    sx, sy, sz = int(sx), int(sy), int(sz)
    ex, ey, ez = int(ex), int(ey), int(ez)
    T_total = int(max_iterations)

    phi0 = np.zeros((CH, GRID, GRID, GRID), np.float32)
    phi0[:, sx, sy, sz] = 1.0

    d_arr = _arrange_D(D + np.float32(0.95))
    smat = _make_smat()

    out = np.empty((T_total, CH, GRID, GRID, GRID), np.float32)
    out[0] = phi0

    state = phi0
    base = 0  # number of steps already recorded beyond frame 0 -> out[0..base]
    # out[k] for k <= base is final. Loop until frozen or all frames produced.
    while base < T_total - 1:
        T = min(T_CHUNK, T_total - 1 - base)
        nc = _build(T)
        ins = {"d_in": d_arr, "phi0": _arrange_state(state), "smat": smat}
        fr = np.asarray(_run_chunk(nc, ins))
        frames = _unarrange_frames(fr)  # frames[j] = state after base+j+1 steps
        # detect freeze: reference freezes once sum(nxt[:, ex, ey, ez]) > 0.01.
        # nxt at global step t is out[t+1]; first such t = t*; frames beyond
        # t*+1 equal frame t*+1.
        sums = frames[:, :, ex, ey, ez].sum(axis=1)  # global steps base+1 .. base+T
        hit = np.nonzero(sums > 0.01)[0]
        if hit.size:
            tstar_plus1 = base + 1 + int(hit[0])  # global index of frozen frame
            n_keep = min(tstar_plus1 - base, T)
            out[base + 1 : base + 1 + n_keep] = frames[:n_keep]
            out[tstar_plus1 + 1 :] = out[tstar_plus1]
            return out
        out[base + 1 : base + 1 + T] = frames
        state = frames[T - 1]
        base += T
    return out


# revision 8
# speedup vs baseline: 1.0947x; 1.0939x over previous
"""flash_wave CA kernel for Trainium2 (Bass/Tile).

Device computes T_CHUNK un-frozen CA steps (fp16) and streams every state to
DRAM. The early-exit freeze of the reference is applied host-side: the first
step t* where the wave reaches (ex,ey,ez) is detected from the returned
frames; frames after t*+1 are provably identical to frame t*+1 (the reference
freezes phi), so the tail is replicated on host and any device frames beyond
t*+1 (computed without freezing) are discarded.

Device layout (single core): partition p = xq*32 + y (xq = x>>3),
free dim = [ch 6][xl 8][z 32] (fp16). Shifts:
  x: free-dim xl windows + quadrant-aligned partition-crossing copies (DVE)
  y: PE shift-matrix matmul (boundary mask baked into the matrix) -> PSUM -> ACT
  z: free-dim windows (DVE)
Clip: pn >= 0 always, so clip(x) = min(x, 1.0), applied once to phi_out
before the shifts (clip commutes with shifts).
"""
import numpy as np

GRID = 32
CH = 6
RING = 16
T_CHUNK = 92

_build_cache = {}


def _build(T):
    if T in _build_cache:
        return _build_cache[T]
    import concourse.bacc as bacc
    import concourse.mybir as mybir
    from concourse.bass import AP
    from concourse.tile import TileContext

    F16 = mybir.dt.float16
    F32 = mybir.dt.float32
    OP = mybir.AluOpType

    nc = bacc.Bacc("TRN2", target_bir_lowering=False, debug=False)
    d_in = nc.dram_tensor("d_in", [128, CH * CH * 256], F16, kind="ExternalInput")
    phi0 = nc.dram_tensor("phi0", [128, CH * 256], F16, kind="ExternalInput")
    smat = nc.dram_tensor("smat", [128, 640], F16, kind="ExternalInput")
    frames = nc.dram_tensor("frames", [T, 128, CH * 256], F16, kind="ExternalOutput")

    D = nc.alloc_sbuf_tensor("D", [128, CH * CH * 256], F16)
    S = nc.alloc_sbuf_tensor("S", [128, 640], F16)
    ring = [nc.alloc_sbuf_tensor(f"ring{i}", [128, CH * 256], F16) for i in range(RING)]
    prod = nc.alloc_sbuf_tensor("prod", [128, CH * CH * 256], F16)
    t3 = nc.alloc_sbuf_tensor("t3", [128, CH * 3 * 256], F16)
    u = nc.alloc_sbuf_tensor("u", [128, CH * 256], F16)
    ta = nc.alloc_sbuf_tensor("ta", [128, CH * 256], F16)
    po = nc.alloc_sbuf_tensor("po", [128, CH * 256], F16)
    cl = nc.alloc_sbuf_tensor("cl", [128, CH * 256], F16)
    ps0 = nc.alloc_psum_tensor("ps0", [128, 256], F32)
    ps1 = nc.alloc_psum_tensor("ps1", [128, 256], F32)
    ps2 = nc.alloc_psum_tensor("ps2", [128, 256], F32)
    ps3 = nc.alloc_psum_tensor("ps3", [128, 256], F32)

    with TileContext(nc):
        nc.sync.dma_start(D[:, :], d_in[:, :])
        nc.sync.dma_start(ring[RING - 1][:, :], phi0[:, :])
        nc.sync.dma_start(S[:, :], smat[:, :])

        D4 = D[:, :].rearrange("p (o i c) -> p o i c", o=CH, i=CH, c=256)
        t34 = t3[:, :].rearrange("p (o j c) -> p o j c", o=CH, j=3, c=256)
        u3 = u[:, :].rearrange("p (o c) -> p o c", o=CH, c=256)
        ta3 = ta[:, :].rearrange("p (o c) -> p o c", o=CH, c=256)
        po3 = po[:, :].rearrange("p (o c) -> p o c", o=CH, c=256)
        prod4 = prod[:, :].rearrange("p (o i c) -> p o i c", o=CH, i=CH, c=256)
        cl4 = cl[:, :].rearrange("p (o xl z) -> p o xl z", o=CH, xl=8, z=GRID)
        po4 = po[:, :].rearrange("p (o xl z) -> p o xl z", o=CH, xl=8, z=GRID)
        cl3 = cl[:, :].rearrange("p (o c) -> p o c", o=CH, c=256)

        for t in range(T):
            prev = ring[(t + RING - 1) % RING]
            nxt = ring[t % RING]
            prev3 = prev[:, :].rearrange("p (i c) -> p i c", i=CH, c=256)
            phi_a = prev3[:, 4:6, :].unsqueeze(1).to_broadcast((128, CH, 2, 256))
            phi_c = prev3[:, 0:4, :].unsqueeze(1).to_broadcast((128, CH, 4, 256))
            # mul over i in {4,5} first: its inputs come from DVE's own z-shift
            # writes, so it overlaps the ACT ch0..3 copies of the previous step
            nc.vector.tensor_tensor(prod4[:, :, 4:6, :], D4[:, :, 4:6, :], phi_a, op=OP.mult)
            nc.vector.tensor_tensor(ta3, prod4[:, :, 4, :], prod4[:, :, 5, :], op=OP.add)
            nc.vector.tensor_tensor(prod4[:, :, 0:4, :], D4[:, :, 0:4, :], phi_c, op=OP.mult)
            nc.vector.tensor_tensor(
                t34[:, :, 0:2, :], prod4[:, :, 0:4:2, :], prod4[:, :, 1:4:2, :], op=OP.add
            )
            nc.vector.tensor_tensor(u3, t34[:, :, 0, :], t34[:, :, 1, :], op=OP.add)
            nc.vector.tensor_tensor(po3, u3, ta3, op=OP.add)
            nc.vector.tensor_scalar_min(cl[:, 0 : 4 * 256], po[:, 0 : 4 * 256], 1.0)

            nxt4 = nxt[:, :].rearrange("p (o xl z) -> p o xl z", o=CH, xl=8, z=GRID)
            zb = AP(nxt, 4 * 256, [[1536, 128], [287, 2], [32, 8]])
            nc.vector.memset(zb, 0.0)
            # ch0 (+x): free offset via identity matmul + quadrant-crossing matrix
            nc.tensor.matmul(ps0[:, 32:256], S[:, 512:640], cl3[:, 0, 0:224], start=True, stop=True)
            nc.tensor.matmul(ps0[:, 0:32], S[:, 256:384], cl3[:, 0, 224:256], start=True, stop=True)
            # ch1 (-x)
            nc.tensor.matmul(ps1[:, 0:224], S[:, 512:640], cl3[:, 1, 32:256], start=True, stop=True)
            nc.tensor.matmul(ps1[:, 224:256], S[:, 384:512], cl3[:, 1, 0:32], start=True, stop=True)
            # ch2 (+y), ch3 (-y)
            nc.tensor.matmul(ps2[:, :], S[:, 0:128], cl3[:, 2, :], start=True, stop=True)
            nc.tensor.matmul(ps3[:, :], S[:, 128:256], cl3[:, 3, :], start=True, stop=True)
            nc.scalar.copy(nxt4[:, 0, 1:8, :].rearrange("p a b -> p (a b)"), ps0[:, 32:256])
            nc.scalar.copy(nxt4[:, 0, 0, :], ps0[:, 0:32])
            nc.scalar.copy(nxt4[:, 1, 0:7, :].rearrange("p a b -> p (a b)"), ps1[:, 0:224])
            nc.scalar.copy(nxt4[:, 1, 7, :], ps1[:, 224:256])
            nc.scalar.copy(nxt4[:, 2, :, :].rearrange("p a b -> p (a b)"), ps2[:, :])
            nc.scalar.copy(nxt4[:, 3, :, :].rearrange("p a b -> p (a b)"), ps3[:, :])
            zout = AP(nxt, 4 * 256 + 1, [[1536, 128], [255, 2], [32, 8], [1, GRID - 1]])
            zin = AP(po, 4 * 256, [[1536, 128], [257, 2], [32, 8], [1, GRID - 1]])
            nc.vector.tensor_scalar_min(zout, zin, 1.0)

            nc.sync.dma_start(frames[t], nxt[:, :])
    nc.compile()
    _build_cache[T] = nc
    return nc


def _arrange_D(Dact):
    a = Dact.reshape(CH, CH, 4, 8, GRID, GRID)  # o i xq xl y z
    a = a.transpose(2, 4, 0, 1, 3, 5).reshape(128, CH * CH * 256)
    return np.ascontiguousarray(a).astype(np.float16)


def _arrange_state(phi):
    a = phi.reshape(CH, 4, 8, GRID, GRID).transpose(1, 3, 0, 2, 4).reshape(128, CH * 256)
    return np.ascontiguousarray(a).astype(np.float16)


def _unarrange_frames(fr):  # [T,128,1536] -> [T,6,32,32,32] float32
    T = fr.shape[0]
    return (
        fr.reshape(T, 4, GRID, CH, 8, GRID)
        .transpose(0, 3, 1, 4, 2, 5)
        .reshape(T, CH, GRID, GRID, GRID)
        .astype(np.float32)
    )


def _make_smat():
    m = np.arange(128)
    S_up = ((m[None, :] - 1 == m[:, None]) & (m[None, :] % 32 != 0)).astype(np.float16)
    S_dn = ((m[None, :] + 1 == m[:, None]) & (m[None, :] % 32 != 31)).astype(np.float16)
    Sx_up = (m[:, None] == m[None, :] - 32).astype(np.float16)
    Sx_dn = (m[:, None] == m[None, :] + 32).astype(np.float16)
    I = np.eye(128, dtype=np.float16)
    return np.concatenate([S_up, S_dn, Sx_up, Sx_dn, I], axis=1)


def _run_chunk(nc, ins, retries=3):
    from concourse.bass_utils import run_bass_kernel_spmd

    last = None
    for _ in range(retries):
        try:
            res = run_bass_kernel_spmd(nc, [ins], core_ids=[0])
            return res.results[0]["frames"]
        except Exception as e:  # transient NRT_EXEC_UNIT_UNRECOVERABLE seen on this setup
            last = e
    raise last


def kernel(D, sx, sy, sz, ex, ey, ez, max_iterations):
    D = np.asarray(D, dtype=np.float32)
    sx, sy, sz = int(sx), int(sy), int(sz)
    ex, ey, ez = int(ex), int(ey), int(ez)
    T_total = int(max_iterations)

    phi0 = np.zeros((CH, GRID, GRID, GRID), np.float32)
    phi0[:, sx, sy, sz] = 1.0

    d_arr = _arrange_D(D + np.float32(0.95))
    smat = _make_smat()

    out = np.empty((T_total, CH, GRID, GRID, GRID), np.float32)
    out[0] = phi0

    state = phi0
    base = 0  # number of steps already recorded beyond frame 0 -> out[0..base]
    # out[k] for k <= base is final. Loop until frozen or all frames produced.
    while base < T_total - 1:
        T = min(T_CHUNK, T_total - 1 - base)
        nc = _build(T)
        ins = {"d_in": d_arr, "phi0": _arrange_state(state), "smat": smat}
        fr = np.asarray(_run_chunk(nc, ins))
        frames = _unarrange_frames(fr)  # frames[j] = state after base+j+1 steps
        # detect freeze: reference freezes once sum(nxt[:, ex, ey, ez]) > 0.01.
        # nxt at global step t is out[t+1]; first such t = t*; frames beyond
        # t*+1 equal frame t*+1.
        sums = frames[:, :, ex, ey, ez].sum(axis=1)  # global steps base+1 .. base+T
        hit = np.nonzero(sums > 0.01)[0]
        if hit.size:
            tstar_plus1 = base + 1 + int(hit[0])  # global index of frozen frame
            n_keep = min(tstar_plus1 - base, T)
            out[base + 1 : base + 1 + n_keep] = frames[:n_keep]
            out[tstar_plus1 + 1 :] = out[tstar_plus1]
            return out
        out[base + 1 : base + 1 + T] = frames
        state = frames[T - 1]
        base += T
    return out


# revision 9
# speedup vs baseline: 1.1426x; 1.0437x over previous
"""flash_wave CA kernel for Trainium2 (Bass/Tile).

Device computes T_CHUNK un-frozen CA steps (fp16) and streams every state to
DRAM. The early-exit freeze of the reference is applied host-side: the first
step t* where the wave reaches (ex,ey,ez) is detected from the returned
frames; frames after t*+1 are provably identical to frame t*+1 (the reference
freezes phi), so the tail is replicated on host and any device frames beyond
t*+1 (computed without freezing) are discarded.

Device layout (single core): partition p = xq*32 + y (xq = x>>3),
free dim = [ch 6][xl 8][z 32] (fp16). Shifts:
  x: free-dim xl windows + quadrant-aligned partition-crossing copies (DVE)
  y: PE shift-matrix matmul (boundary mask baked into the matrix) -> PSUM -> ACT
  z: free-dim windows (DVE)
Clip: pn >= 0 always, so clip(x) = min(x, 1.0), applied once to phi_out
before the shifts (clip commutes with shifts).
"""
import numpy as np

GRID = 32
CH = 6
RING = 16
T_CHUNK = 88

_build_cache = {}


def _build(T):
    if T in _build_cache:
        return _build_cache[T]
    import concourse.bacc as bacc
    import concourse.mybir as mybir
    from concourse.bass import AP
    from concourse.tile import TileContext

    F16 = mybir.dt.float16
    F32 = mybir.dt.float32
    OP = mybir.AluOpType

    nc = bacc.Bacc("TRN2", target_bir_lowering=False, debug=False)
    d_in = nc.dram_tensor("d_in", [128, CH * CH * 256], F16, kind="ExternalInput")
    phi0 = nc.dram_tensor("phi0", [128, CH * 256], F16, kind="ExternalInput")
    smat = nc.dram_tensor("smat", [128, 640], F16, kind="ExternalInput")
    frames = nc.dram_tensor("frames", [T, 128, CH * 256], F16, kind="ExternalOutput")

    D = nc.alloc_sbuf_tensor("D", [128, CH * CH * 256], F16)
    S = nc.alloc_sbuf_tensor("S", [128, 640], F16)
    ring = [nc.alloc_sbuf_tensor(f"ring{i}", [128, CH * 256], F16) for i in range(RING)]
    prod = nc.alloc_sbuf_tensor("prod", [128, CH * CH * 256], F16)
    t3 = nc.alloc_sbuf_tensor("t3", [128, CH * 3 * 256], F16)
    u = nc.alloc_sbuf_tensor("u", [128, CH * 256], F16)
    ta = nc.alloc_sbuf_tensor("ta", [128, CH * 256], F16)
    po = nc.alloc_sbuf_tensor("po", [128, CH * 256], F16)
    cl = nc.alloc_sbuf_tensor("cl", [128, CH * 256], F16)
    ps0 = nc.alloc_psum_tensor("ps0", [128, 256], F32)
    ps1 = nc.alloc_psum_tensor("ps1", [128, 256], F32)
    ps2 = nc.alloc_psum_tensor("ps2", [128, 256], F32)
    ps3 = nc.alloc_psum_tensor("ps3", [128, 256], F32)

    with TileContext(nc):
        nc.sync.dma_start(D[:, :], d_in[:, :])
        nc.sync.dma_start(ring[RING - 1][:, :], phi0[:, :])
        nc.sync.dma_start(S[:, :], smat[:, :])

        D4 = D[:, :].rearrange("p (o i c) -> p o i c", o=CH, i=CH, c=256)
        t34 = t3[:, :].rearrange("p (o j c) -> p o j c", o=CH, j=3, c=256)
        u3 = u[:, :].rearrange("p (o c) -> p o c", o=CH, c=256)
        ta3 = ta[:, :].rearrange("p (o c) -> p o c", o=CH, c=256)
        po3 = po[:, :].rearrange("p (o c) -> p o c", o=CH, c=256)
        prod4 = prod[:, :].rearrange("p (o i c) -> p o i c", o=CH, i=CH, c=256)
        cl4 = cl[:, :].rearrange("p (o xl z) -> p o xl z", o=CH, xl=8, z=GRID)
        po4 = po[:, :].rearrange("p (o xl z) -> p o xl z", o=CH, xl=8, z=GRID)
        cl3 = cl[:, :].rearrange("p (o c) -> p o c", o=CH, c=256)

        for t in range(T):
            prev = ring[(t + RING - 1) % RING]
            nxt = ring[t % RING]
            prev3 = prev[:, :].rearrange("p (i c) -> p i c", i=CH, c=256)
            phi_a = prev3[:, 4:6, :].unsqueeze(1).to_broadcast((128, CH, 2, 256))
            phi_c = prev3[:, 0:4, :].unsqueeze(1).to_broadcast((128, CH, 4, 256))
            # mul over i in {4,5} first: its inputs come from DVE's own z-shift
            # writes, so it overlaps the ACT ch0..3 copies of the previous step
            nc.vector.tensor_tensor(prod4[:, :, 4:6, :], D4[:, :, 4:6, :], phi_a, op=OP.mult)
            nc.vector.tensor_tensor(ta3, prod4[:, :, 4, :], prod4[:, :, 5, :], op=OP.add)
            nc.vector.tensor_tensor(prod4[:, :, 0:4, :], D4[:, :, 0:4, :], phi_c, op=OP.mult)
            nc.vector.tensor_tensor(
                t34[:, :, 0:2, :], prod4[:, :, 0:4:2, :], prod4[:, :, 1:4:2, :], op=OP.add
            )
            nc.vector.tensor_tensor(u3, t34[:, :, 0, :], t34[:, :, 1, :], op=OP.add)
            nc.vector.tensor_tensor(po3, u3, ta3, op=OP.add)
            nc.vector.tensor_scalar_min(cl[:, 0 : 4 * 256], po[:, 0 : 4 * 256], 1.0)

            nxt4 = nxt[:, :].rearrange("p (o xl z) -> p o xl z", o=CH, xl=8, z=GRID)
            zb = AP(nxt, 4 * 256, [[1536, 128], [287, 2], [32, 8]])
            nc.vector.memset(zb, 0.0)
            # ch0 (+x): free offset via identity matmul + quadrant-crossing matrix
            nc.tensor.matmul(ps0[:, 32:256], S[:, 512:640], cl3[:, 0, 0:224], start=True, stop=True)
            nc.tensor.matmul(ps0[:, 0:32], S[:, 256:384], cl3[:, 0, 224:256], start=True, stop=True)
            # ch1 (-x)
            nc.tensor.matmul(ps1[:, 0:224], S[:, 512:640], cl3[:, 1, 32:256], start=True, stop=True)
            nc.tensor.matmul(ps1[:, 224:256], S[:, 384:512], cl3[:, 1, 0:32], start=True, stop=True)
            # ch2 (+y), ch3 (-y)
            nc.tensor.matmul(ps2[:, :], S[:, 0:128], cl3[:, 2, :], start=True, stop=True)
            nc.tensor.matmul(ps3[:, :], S[:, 128:256], cl3[:, 3, :], start=True, stop=True)
            nc.scalar.copy(nxt4[:, 0, 1:8, :].rearrange("p a b -> p (a b)"), ps0[:, 32:256])
            nc.scalar.copy(nxt4[:, 0, 0, :], ps0[:, 0:32])
            nc.scalar.copy(nxt4[:, 1, 0:7, :].rearrange("p a b -> p (a b)"), ps1[:, 0:224])
            nc.scalar.copy(nxt4[:, 1, 7, :], ps1[:, 224:256])
            nc.scalar.copy(nxt4[:, 2, :, :].rearrange("p a b -> p (a b)"), ps2[:, :])
            nc.scalar.copy(nxt4[:, 3, :, :].rearrange("p a b -> p (a b)"), ps3[:, :])
            zout = AP(nxt, 4 * 256 + 1, [[1536, 128], [255, 2], [32, 8], [1, GRID - 1]])
            zin = AP(po, 4 * 256, [[1536, 128], [257, 2], [32, 8], [1, GRID - 1]])
            nc.vector.tensor_scalar_min(zout, zin, 1.0)

            nc.sync.dma_start(frames[t], nxt[:, :])
    nc.compile()
    _build_cache[T] = nc
    return nc


def _arrange_D(Dact):
    a = Dact.reshape(CH, CH, 4, 8, GRID, GRID)  # o i xq xl y z
    a = a.transpose(2, 4, 0, 1, 3, 5).reshape(128, CH * CH * 256)
    return np.ascontiguousarray(a).astype(np.float16)


def _arrange_state(phi):
    a = phi.reshape(CH, 4, 8, GRID, GRID).transpose(1, 3, 0, 2, 4).reshape(128, CH * 256)
    return np.ascontiguousarray(a).astype(np.float16)


def _unarrange_frames(fr):  # [T,128,1536] -> [T,6,32,32,32] float32
    T = fr.shape[0]
    return (
        fr.reshape(T, 4, GRID, CH, 8, GRID)
        .transpose(0, 3, 1, 4, 2, 5)
        .reshape(T, CH, GRID, GRID, GRID)
        .astype(np.float32)
    )


def _make_smat():
    m = np.arange(128)
    S_up = ((m[None, :] - 1 == m[:, None]) & (m[None, :] % 32 != 0)).astype(np.float16)
    S_dn = ((m[None, :] + 1 == m[:, None]) & (m[None, :] % 32 != 31)).astype(np.float16)
    Sx_up = (m[:, None] == m[None, :] - 32).astype(np.float16)
    Sx_dn = (m[:, None] == m[None, :] + 32).astype(np.float16)
    I = np.eye(128, dtype=np.float16)
    return np.concatenate([S_up, S_dn, Sx_up, Sx_dn, I], axis=1)


def _run_chunk(nc, ins, retries=3):
    from concourse.bass_utils import run_bass_kernel_spmd

    last = None
    for _ in range(retries):
        try:
            res = run_bass_kernel_spmd(nc, [ins], core_ids=[0])
            return res.results[0]["frames"]
        except Exception as e:  # transient NRT_EXEC_UNIT_UNRECOVERABLE seen on this setup
            last = e
    raise last


def kernel(D, sx, sy, sz, ex, ey, ez, max_iterations):
    D = np.asarray(D, dtype=np.float32)
    sx, sy, sz = int(sx), int(sy), int(sz)
    ex, ey, ez = int(ex), int(ey), int(ez)
    T_total = int(max_iterations)

    phi0 = np.zeros((CH, GRID, GRID, GRID), np.float32)
    phi0[:, sx, sy, sz] = 1.0

    d_arr = _arrange_D(D + np.float32(0.95))
    smat = _make_smat()

    out = np.empty((T_total, CH, GRID, GRID, GRID), np.float32)
    out[0] = phi0

    state = phi0
    base = 0  # number of steps already recorded beyond frame 0 -> out[0..base]
    # out[k] for k <= base is final. Loop until frozen or all frames produced.
    while base < T_total - 1:
        T = min(T_CHUNK, T_total - 1 - base)
        nc = _build(T)
        ins = {"d_in": d_arr, "phi0": _arrange_state(state), "smat": smat}
        fr = np.asarray(_run_chunk(nc, ins))
        frames = _unarrange_frames(fr)  # frames[j] = state after base+j+1 steps
        # detect freeze: reference freezes once sum(nxt[:, ex, ey, ez]) > 0.01.
        # nxt at global step t is out[t+1]; first such t = t*; frames beyond
        # t*+1 equal frame t*+1.
        sums = frames[:, :, ex, ey, ez].sum(axis=1)  # global steps base+1 .. base+T
        hit = np.nonzero(sums > 0.01)[0]
        if hit.size:
            tstar_plus1 = base + 1 + int(hit[0])  # global index of frozen frame
            n_keep = min(tstar_plus1 - base, T)
            out[base + 1 : base + 1 + n_keep] = frames[:n_keep]
            out[tstar_plus1 + 1 :] = out[tstar_plus1]
            return out
        out[base + 1 : base + 1 + T] = frames
        state = frames[T - 1]
        base += T
    return out
